# revision 67
# baseline (speedup 1.0000x reference)
"""Ensemble-MLP (grouped 1x1 conv) Trainium2 kernel.

Computation (per batch row b):
  h = relu(x @ W0[e] + b0[e])             e = 0..9 ensembles, 256 units
  h = relu(h @ Wh[l,e] + bh[l,e])         l = 0..6 hidden layers
  y[e] = h @ Wf[e] + bf[e]                201 outputs per ensemble
  out[b, o'] = mean_j yflat[b, o'*10 + j] (strided channel mix, yflat = e*201+o)

Strategy (v3):
  * Data parallel: batch 16384 -> 2048 rows per core on 8 cores. Weights
    replicated, no collectives.
  * bf16 operands on the PE (fp8 blows the 2e-2 error budget: ~15% measured);
    fp32 PSUM accumulation.
  * Activations live transposed in SBUF: H[channel, batch], 2 chunks of 128
    channels x 2048 batch. Every layer: matmul(psum[o,b] += W[c,o].T @ H[c,b]).
  * Layer-0 bias folded into the matmul via an all-ones row on x^T (K=7).
    x ships as a tiny (8, 2048) tensor and is quad-replicated on-chip into
    partition offsets 0/32/64/96 so L0's K=7 matmuls run 4-concurrent in the
    PE row groups. Hidden biases ride the relu post-op.
  * L0 of ensemble e+1 is issued BEFORE the hidden layers of ensemble e:
    relu/semaphore latency at every layer-0 boundary is hidden behind PE work.
  * All DMAs ride the sync+scalar HW-DGE queues. gpsimd issues nothing:
    its end-of-kernel software-DGE drain (~8us) disappears.
  * Weight DMAs are issued two ensembles ahead (triple-buffered) so
    LDWEIGHTS never waits on the weight queue.
  * Final channel-mixing mean folded into the last-layer weights on the host
    (exact linear algebra). Layer-7 activations for all 10 ensembles are kept
    in SBUF (bf16, 10MB) and the ensemble sum accumulates IN PSUM (20 matmuls
    per bank slice). Bias-add split 256/256 across ACT+DVE, stores split
    across the scalar/sync queues, so the post-matmul tail is minimal.
  * PE pre-warm: dummy matmuls on a memset scratch tile right after the
    framework preamble so the DVFS ramp starts before real work.
"""

import numpy as np
from contextlib import ExitStack

import ml_dtypes
import concourse.bass as bass
import concourse.mybir as mybir
import concourse.tile as tile
from concourse import bacc, bass_utils

F32 = mybir.dt.float32
BF16 = mybir.dt.bfloat16
BF16_NP = ml_dtypes.bfloat16

ENS, N_UNITS, N_HID, IN_DIM, OUT_DIM, BATCH = 10, 256, 7, 6, 201, 16384
N_CORES = 8
BC = BATCH // N_CORES          # 2048 batch rows per core
N_WARM = 9                     # pre-warm matmuls for the DVFS ramp

_CACHE = {}


def build_program():
    nc = bacc.Bacc("TRN2", debug=False)

    # Every [128, *] DMA costs ~4.2us in descriptor generation (128 rows x
    # ~33ns) regardless of size, so tensors are MERGED into as few DMAs as
    # possible: xw ships x + all L0 weights on 8 partitions (8 descriptors!)
    # and is quad-replicated on-chip; whb carries a whole ensemble's hidden
    # weights + biases (bf16) in one DMA; vwp carries the final weights +
    # bias in one DMA.
    XW_COLS = BC + ENS * 128
    xw = nc.dram_tensor("xw", (128, XW_COLS), BF16, kind="ExternalInput").ap()
    # biases ride along as raw f32 bytes (2 bf16 slots each), bitcast on read
    whb = nc.dram_tensor("whb", (ENS, 128, N_HID * 512 + N_HID * 4), BF16,
                         kind="ExternalInput").ap()
    vwp = nc.dram_tensor("vwp", (128, ENS * 512 + 4), BF16,
                         kind="ExternalInput").ap()
    yt = nc.dram_tensor("yt", (256, BC), F32, kind="ExternalOutput").ap()

    add = mybir.AluOpType.add
    mx = mybir.AluOpType.max
    relu = mybir.ActivationFunctionType.Relu
    ident = mybir.ActivationFunctionType.Identity

    with ExitStack() as ctx:
        tc = ctx.enter_context(tile.TileContext(nc))
        const = ctx.enter_context(tc.tile_pool(name="const", bufs=1))
        wpool = ctx.enter_context(tc.tile_pool(name="w", bufs=4))
        vpool = ctx.enter_context(tc.tile_pool(name="v", bufs=1))
        l0pool = ctx.enter_context(tc.tile_pool(name="l0", bufs=2))
        hpool = ctx.enter_context(tc.tile_pool(name="h", bufs=2))
        hfpool = ctx.enter_context(tc.tile_pool(name="hf", bufs=2 * ENS))
        spool = ctx.enter_context(tc.tile_pool(name="stage", bufs=3))
        pspool = ctx.enter_context(tc.tile_pool(name="ps", bufs=8, space="PSUM"))

        x_t = const.tile([128, XW_COLS], BF16)
        scratch = const.tile([128, 512], BF16)
        v_all = vpool.tile([128, ENS * 512 + 4], BF16)

        wh_t = {}
        l0_out, hf = {}, {}

        # memset on gpsimd: vector/scalar are stuck behind their framework
        # table-load DMAs at startup; gpsimd is free ~1us earlier. gpsimd
        # issues no DMAs, so no software-DGE drain at kernel end.
        nc.gpsimd.memset(scratch, 0.0)

        # whb column layout (per ensemble, built on host):
        #   [l0-3 weights 2048 | l0-3 biases 16 | l4-6 weights 1536 |
        #    l4-6 biases 12]  -> 3612 bf16 cols; biases are raw f32 bytes.
        WH_COLS = N_HID * 512 + N_HID * 4
        SPL = 4 * 512 + 4 * 4  # 2064: start of the l4-6 half

        def issue_wh(e, eng):
            wh_t[e] = wpool.tile([128, WH_COLS], BF16,
                                 tag="wh", name=f"wh_e{e}")
            eng.dma_start(out=wh_t[e], in_=whb[e])

        # ensemble 0's weights are on the cold-start critical path: split
        # them over BOTH queues as two separate tiles (layers 0-3 via
        # scalar, 4-6 via sync behind xw) so no single queue's latency
        # jitter can stall hidden(0).
        wh0a = const.tile([128, SPL], BF16)
        wh0b = const.tile([128, WH_COLS - SPL], BF16)

        def _part(e, l):
            if e == 0:
                return (wh0a, 0) if l < 4 else (wh0b, SPL)
            return wh_t[e], 0

        def wh_ap(e, l):
            """(tile, weight column base) for hidden layer l of ensemble e."""
            t, off = _part(e, l)
            base = l * 512 if l < 4 else SPL + (l - 4) * 512
            return t, base - off

        def bh_ap(e, l, oc):
            t, off = _part(e, l)
            bb = (4 * 512 + l * 4 if l < 4
                  else SPL + 3 * 512 + (l - 4) * 4) + 2 * oc - off
            return t[:, bb:bb + 2].bitcast(F32)

        # startup: ONE sync DMA carries x (quad-replicated on host) plus all
        # ten ensembles' L0 weights; the scalar queue concurrently streams
        # e0/e1 hidden weights. Every DMA pays ~4-5us of fixed queue+
        # descriptor latency, so fewer/bigger transfers win.
        nc.sync.dma_start(out=x_t, in_=xw)
        nc.scalar.dma_start(out=wh0a, in_=whb[0][:, :SPL])
        nc.sync.dma_start(out=wh0b, in_=whb[0][:, SPL:])
        issue_wh(1, nc.scalar)
        issue_wh(2, nc.sync)

        # ---- PE pre-warm: dummy matmuls on zeroed scratch, result unread ----
        for k in range(N_WARM):
            ps_warm = pspool.tile([128, 512], F32, tag="ps", name=f"warm{k}")
            nc.tensor.matmul(ps_warm, lhsT=scratch[:, 0:128],
                             rhs=scratch, start=True, stop=True)

        def relu_tile(engine_is_act, dst, ps, bias_ap):
            if engine_is_act:
                nc.scalar.activation(out=dst, in_=ps, func=relu,
                                     bias=bias_ap if bias_ap is not None else 0.0)
            elif bias_ap is not None:
                nc.vector.tensor_scalar(out=dst, in0=ps, scalar1=bias_ap,
                                        scalar2=0.0, op0=add, op1=mx)
            else:
                nc.vector.tensor_scalar(out=dst, in0=ps, scalar1=0.0,
                                        scalar2=None, op0=mx)

        def issue_l0(e):
            # x^T (7, BC) -> h (2x128, BC); bias folded in. 4 K=7 matmuls run
            # concurrently in the 4 PE row groups (row-group i holds weights
            # for oc=i//2, streams bt parity i%2). Relus per 256-col slice,
            # alternated ACT/DVE.
            l0_out[e] = [l0pool.tile([128, BC], BF16, tag=f"l0_{kc}",
                                     name=f"l0_{kc}_e{e}")
                         for kc in range(2)]
            for j in range(2):
                pst = {}
                for i in range(4):
                    p = i % 2
                    bt = 2 * j + p
                    pst[i] = pspool.tile([128, 512], F32, tag="ps",
                                         name=f"ps{i}_{j}_e{e}L0")
                    w0c = BC + e * 128
                    nc.tensor.matmul(
                        pst[i],
                        lhsT=x_t[32 * i:32 * i + IN_DIM + 1, w0c:w0c + 128],
                        rhs=x_t[32 * i:32 * i + IN_DIM + 1,
                                bt * 512:(bt + 1) * 512],
                        start=True, stop=True, tile_position=(32 * i, 0))
                for i in range(4):
                    oc, p = i // 2, i % 2
                    bt = 2 * j + p
                    for s in range(2):
                        c0 = bt * 512 + s * 256
                        relu_tile(s == 0 and i < 2,
                                  l0_out[e][oc][:, c0:c0 + 256],
                                  pst[i][:, s * 256:(s + 1) * 256], None)

        issue_l0(0)
        # filler warms: bridge the gap between L0(0) finishing and e0's
        # hidden weights landing, so the DVFS ramp never resets.
        for k in range(3):
            ps_fill = pspool.tile([128, 512], F32, tag="ps", name=f"fill{k}")
            nc.tensor.matmul(ps_fill, lhsT=scratch[:, 0:128],
                             rhs=scratch, start=True, stop=True)

        def issue_hidden(e, h_cur, l_lo, l_hi):
            # hidden layers [l_lo, l_hi): K=256 (2 chunks), M=256 (2 chunks)
            for l in range(l_lo, l_hi):
                if l < N_HID - 1:
                    h_nxt = [hpool.tile([128, BC], BF16, tag=f"h{kc}",
                                        name=f"h{kc}_e{e}l{l}")
                             for kc in range(2)]
                else:
                    h_nxt = [hfpool.tile([128, BC], BF16, tag="hf",
                                         name=f"hf{e}_{kc}")
                             for kc in range(2)]
                    for kc in range(2):
                        hf[(e, kc)] = h_nxt[kc]
                wt, base = wh_ap(e, l)
                eng = 0
                for bt in range(4):
                    hsl = slice(bt * 512, (bt + 1) * 512)
                    for oc in range(2):
                        ps = pspool.tile([128, 512], F32, tag="ps",
                                         name=f"ps{oc}_{bt}_e{e}l{l}")
                        c0 = base + oc * 128
                        c1 = base + N_UNITS + oc * 128
                        nc.tensor.matmul(ps, lhsT=wt[:, c0:c0 + 128],
                                         rhs=h_cur[0][:, hsl],
                                         start=True, stop=False)
                        nc.tensor.matmul(ps, lhsT=wt[:, c1:c1 + 128],
                                         rhs=h_cur[1][:, hsl],
                                         start=False, stop=True)
                        # ACT runs ~15% slower per op than DVE and also
                        # carries the L0 relus: give it 3 of 8 slices, and
                        # keep bt0 (whose relus gate the next layer's first
                        # matmuls) on the lighter engine.
                        relu_tile(eng in (0, 1, 4), h_nxt[oc][:, hsl], ps,
                                  bh_ap(e, l, oc))
                        eng += 1
                h_cur = h_nxt
            return h_cur

        for e in range(ENS):
            if e + 3 < ENS:
                # prefetch three deep, alternating queues by parity, to
                # absorb the per-DMA queue latency.
                issue_wh(e + 3, nc.scalar if (e + 3) % 2 else nc.sync)
            if e == 1:
                nc.sync.dma_start(out=v_all, in_=vwp)
            # L0(e+1) is issued BETWEEN hidden layers 4 and 5 of ensemble e:
            # its relus then order ahead of the tail-layer relus in the
            # ACT/DVE queues, avoiding a priority inversion where the PE
            # stalls on an L0-relu PSUM release queued behind an l6 relu.
            h_cur = issue_hidden(e, l0_out.pop(e), 0, N_HID - 2)
            if e + 1 < ENS:
                issue_l0(e + 1)
            issue_hidden(e, h_cur, N_HID - 2, N_HID)

        # ---- final layer: out[o', b] = sum_e sum_kc V[e][kc].T @ hf[e][kc] ----
        # Ensemble sum accumulates in PSUM (20 matmuls per bank). Bias-add is
        # split 256/256 across ACT+DVE; stores split across scalar/sync.
        for g, (bt, oc) in enumerate([(bt, oc) for bt in range(4)
                                      for oc in range(2)]):
            ps = pspool.tile([128, 512], F32, tag="ps", name=f"psf{g}")
            hsl = slice(bt * 512, (bt + 1) * 512)
            for e in range(ENS):
                for kc in range(2):
                    c = e * 512 + kc * 256 + oc * 128
                    nc.tensor.matmul(ps, lhsT=v_all[:, c:c + 128],
                                     rhs=hf[(e, kc)][:, hsl],
                                     start=(e == 0 and kc == 0),
                                     stop=(e == ENS - 1 and kc == 1))
            stage = spool.tile([128, 512], F32, tag="s", name=f"stage{g}")
            bpc = ENS * 512 + 2 * oc
            bp_ap = v_all[:, bpc:bpc + 2].bitcast(F32)
            nc.scalar.activation(out=stage[:, 0:256], in_=ps[:, 0:256],
                                 func=ident, bias=bp_ap)
            nc.vector.tensor_scalar(out=stage[:, 256:512], in0=ps[:, 256:512],
                                    scalar1=bp_ap,
                                    scalar2=None, op0=add)
            # partition-split stores: descriptor count (and so DMA latency)
            # scales with partition rows, and the LAST store's latency sits
            # on the kernel's critical path before the exit drain.
            nc.scalar.dma_start(out=yt[oc * 128:oc * 128 + 64, hsl],
                                in_=stage[0:64, :])
            nc.sync.dma_start(out=yt[oc * 128 + 64:(oc + 1) * 128, hsl],
                              in_=stage[64:128, :])

    nc.compile()
    return nc


def prepare_inputs(x, W0, b0, Wh, bh, Wf, bf):
    """Host-side weight refactoring + per-core sharding. Exact fp32 linear
    algebra for the folds; bf16 quantization only at the very end."""
    x = np.asarray(x, np.float32)
    W0 = np.asarray(W0, np.float32)
    b0 = np.asarray(b0, np.float32)
    Wh = np.asarray(Wh, np.float32)
    bh = np.asarray(bh, np.float32)
    Wf = np.asarray(Wf, np.float32)
    bf = np.asarray(bf, np.float32)

    # layer 0 with bias folded: lhsT rows = 6 inputs + ones row; packed into
    # the 4 PE row groups (groups 0,1 -> oc0 weights; groups 2,3 -> oc1).
    w0a = np.concatenate([W0, b0[:, None, :]], axis=1)  # (ENS, 7, 256)
    w0q = np.zeros((128, ENS, 128), np.float32)
    for i in range(4):
        w0q[32 * i:32 * i + IN_DIM + 1] = \
            w0a[:, :, (i // 2) * 128:(i // 2) * 128 + 128].transpose(1, 0, 2)
    w0q = w0q.reshape(128, ENS * 128)

    # hidden weights -> [e, p, (l, kc, o)] with the biases [e, p, (l, oc)]
    # (as bf16) appended so each ensemble is ONE dma.
    whh = (Wh.transpose(1, 0, 2, 3)              # (e, l, h, o)
             .reshape(ENS, N_HID, 2, 128, N_UNITS)
             .transpose(0, 3, 1, 2, 4)           # (e, p, l, kc, o)
             .reshape(ENS, 128, N_HID * 2 * N_UNITS))
    bhh = (bh.transpose(1, 0, 2)                 # (e, l, o)
             .reshape(ENS, N_HID, 2, 128)
             .transpose(0, 3, 1, 2)              # (e, p, l, oc)
             .reshape(ENS, 128, N_HID * 2))
    whh16 = np.ascontiguousarray(whh).astype(BF16_NP).view('<u2')
    bhh16 = np.ascontiguousarray(bhh.astype('<f4')).view('<u2')  # raw bytes
    # [l0-3 w | l0-3 biases | l4-6 w | l4-6 biases] so ensemble 0 can split
    # into two tiles with each half carrying its own biases
    whb = np.concatenate([whh16[:, :, :4 * 512], bhh16[:, :, :16],
                          whh16[:, :, 4 * 512:], bhh16[:, :, 16:]],
                         axis=2)                 # (ENS, 128, 3612) u2
    whb = np.ascontiguousarray(whb).view(BF16_NP)

    # fold the strided channel-mix mean into the final weights:
    # out[b, o'] = 0.1 * sum_j yflat[b, o'*10+j],  yflat col c = e*201+o
    C = ENS * OUT_DIM
    M = np.zeros((C, OUT_DIM), np.float32)
    M[np.arange(C), np.arange(C) // ENS] = 1.0 / ENS
    Me = M.reshape(ENS, OUT_DIM, OUT_DIM)
    V = np.einsum('eho,eoc->ehc', Wf, Me)        # (ENS, 256, 201)
    bpv = bf.reshape(C) @ M                      # (201,)

    Vp = np.zeros((ENS, N_UNITS, 256), np.float32)
    Vp[:, :, :OUT_DIM] = V
    vww = (Vp.reshape(ENS, 2, 128, 256)
             .transpose(0, 2, 1, 3)              # (e, p, kc, o')
             .reshape(ENS, 128, 2 * 256)
             .transpose(1, 0, 2)
             .reshape(128, ENS * 512))
    bp_pad = np.zeros(256, np.float32)
    bp_pad[:OUT_DIM] = bpv
    bp_t = np.ascontiguousarray(bp_pad.reshape(2, 128).T.astype('<f4'))
    vwp = np.concatenate(
        [np.ascontiguousarray(vww).astype(BF16_NP).view('<u2'),
         bp_t.view('<u2')], axis=1)              # (128, 5124) u2
    vwp = np.ascontiguousarray(vwp).view(BF16_NP)

    ones = np.ones((1, BC), np.float32)
    in_maps = []
    for c in range(N_CORES):
        xs = x[c * BC:(c + 1) * BC]              # (BC, 6)
        xt = np.concatenate([xs.T, ones], axis=0)  # (7, BC)
        xrep = np.zeros((128, BC), np.float32)
        for i in range(4):
            xrep[32 * i:32 * i + IN_DIM + 1, :] = xt
        xwc = np.concatenate([xrep, w0q], axis=1)  # (128, BC + 1280)
        xwc = np.ascontiguousarray(xwc).astype(BF16_NP)
        in_maps.append({"xw": xwc, "whb": whb, "vwp": vwp})
    return in_maps


def run(in_maps, trace=False, tmpdir=None):
    if "nc" not in _CACHE:
        _CACHE["nc"] = build_program()
    nc = _CACHE["nc"]
    res = bass_utils.run_bass_kernel_spmd(
        nc, in_maps, core_ids=list(range(N_CORES)), trace=trace, tmpdir=tmpdir)
    return res


def kernel(x, W0, b0, Wh, bh, Wf, bf):
    in_maps = prepare_inputs(x, W0, b0, Wh, bh, Wf, bf)
    res = run(in_maps)
    out = np.empty((BATCH, OUT_DIM), np.float32)
    for c in range(N_CORES):
        out[c * BC:(c + 1) * BC, :] = res.results[c]["yt"][:OUT_DIM].T
    return out


# revision 68
# speedup vs baseline: 1.0410x; 1.0410x over previous
"""Ensemble-MLP (grouped 1x1 conv) Trainium2 kernel.

Computation (per batch row b):
  h = relu(x @ W0[e] + b0[e])             e = 0..9 ensembles, 256 units
  h = relu(h @ Wh[l,e] + bh[l,e])         l = 0..6 hidden layers
  y[e] = h @ Wf[e] + bf[e]                201 outputs per ensemble
  out[b, o'] = mean_j yflat[b, o'*10 + j] (strided channel mix, yflat = e*201+o)

Strategy (v3):
  * Data parallel: batch 16384 -> 2048 rows per core on 8 cores. Weights
    replicated, no collectives.
  * bf16 operands on the PE (fp8 blows the 2e-2 error budget: ~15% measured);
    fp32 PSUM accumulation.
  * Activations live transposed in SBUF: H[channel, batch], 2 chunks of 128
    channels x 2048 batch. Every layer: matmul(psum[o,b] += W[c,o].T @ H[c,b]).
  * Layer-0 bias folded into the matmul via an all-ones row on x^T (K=7).
    x ships as a tiny (8, 2048) tensor and is quad-replicated on-chip into
    partition offsets 0/32/64/96 so L0's K=7 matmuls run 4-concurrent in the
    PE row groups. Hidden biases ride the relu post-op.
  * L0 of ensemble e+1 is issued BEFORE the hidden layers of ensemble e:
    relu/semaphore latency at every layer-0 boundary is hidden behind PE work.
  * All DMAs ride the sync+scalar HW-DGE queues. gpsimd issues nothing:
    its end-of-kernel software-DGE drain (~8us) disappears.
  * Weight DMAs are issued two ensembles ahead (triple-buffered) so
    LDWEIGHTS never waits on the weight queue.
  * Final channel-mixing mean folded into the last-layer weights on the host
    (exact linear algebra). Layer-7 activations for all 10 ensembles are kept
    in SBUF (bf16, 10MB) and the ensemble sum accumulates IN PSUM (20 matmuls
    per bank slice). Bias-add split 256/256 across ACT+DVE, stores split
    across the scalar/sync queues, so the post-matmul tail is minimal.
  * PE pre-warm: dummy matmuls on a memset scratch tile right after the
    framework preamble so the DVFS ramp starts before real work.
"""

import numpy as np
from contextlib import ExitStack

import ml_dtypes
import concourse.bass as bass
import concourse.mybir as mybir
import concourse.tile as tile
from concourse import bacc, bass_utils

F32 = mybir.dt.float32
BF16 = mybir.dt.bfloat16
BF16_NP = ml_dtypes.bfloat16

ENS, N_UNITS, N_HID, IN_DIM, OUT_DIM, BATCH = 10, 256, 7, 6, 201, 16384
N_CORES = 8
BC = BATCH // N_CORES          # 2048 batch rows per core
N_WARM = 9                     # pre-warm matmuls for the DVFS ramp

_CACHE = {}


def build_program():
    nc = bacc.Bacc("TRN2", debug=False)

    # Every [128, *] DMA costs ~4.2us in descriptor generation (128 rows x
    # ~33ns) regardless of size, so tensors are MERGED into as few DMAs as
    # possible: xw ships x + all L0 weights on 8 partitions (8 descriptors!)
    # and is quad-replicated on-chip; whb carries a whole ensemble's hidden
    # weights + biases (bf16) in one DMA; vwp carries the final weights +
    # bias in one DMA.
    XW_COLS = BC + ENS * 128
    xw = nc.dram_tensor("xw", (128, XW_COLS), BF16, kind="ExternalInput").ap()
    # biases ride along as raw f32 bytes (2 bf16 slots each), bitcast on read
    whb = nc.dram_tensor("whb", (ENS, 128, N_HID * 512 + N_HID * 4), BF16,
                         kind="ExternalInput").ap()
    vwp = nc.dram_tensor("vwp", (128, ENS * 512 + 4), BF16,
                         kind="ExternalInput").ap()
    yt = nc.dram_tensor("yt", (256, BC), F32, kind="ExternalOutput").ap()

    add = mybir.AluOpType.add
    mx = mybir.AluOpType.max
    relu = mybir.ActivationFunctionType.Relu
    ident = mybir.ActivationFunctionType.Identity

    with ExitStack() as ctx:
        tc = ctx.enter_context(tile.TileContext(nc))
        const = ctx.enter_context(tc.tile_pool(name="const", bufs=1))
        wpool = ctx.enter_context(tc.tile_pool(name="w", bufs=4))
        vpool = ctx.enter_context(tc.tile_pool(name="v", bufs=1))
        l0pool = ctx.enter_context(tc.tile_pool(name="l0", bufs=2))
        hpool = ctx.enter_context(tc.tile_pool(name="h", bufs=2))
        hfpool = ctx.enter_context(tc.tile_pool(name="hf", bufs=2 * ENS))
        spool = ctx.enter_context(tc.tile_pool(name="stage", bufs=3))
        pspool = ctx.enter_context(tc.tile_pool(name="ps", bufs=8, space="PSUM"))

        x_t = const.tile([128, XW_COLS], BF16)
        scratch = const.tile([128, 512], BF16)
        v_all = vpool.tile([128, ENS * 512 + 4], BF16)

        wh_t = {}
        l0_out, hf = {}, {}

        # memset on gpsimd: vector/scalar are stuck behind their framework
        # table-load DMAs at startup; gpsimd is free ~1us earlier. gpsimd
        # issues no DMAs, so no software-DGE drain at kernel end.
        nc.gpsimd.memset(scratch, 0.0)

        # whb column layout (per ensemble, built on host):
        #   [l0-3 weights 2048 | l0-3 biases 16 | l4-6 weights 1536 |
        #    l4-6 biases 12]  -> 3612 bf16 cols; biases are raw f32 bytes.
        WH_COLS = N_HID * 512 + N_HID * 4
        SPL = 4 * 512 + 4 * 4  # 2064: start of the l4-6 half

        def issue_wh(e, eng):
            wh_t[e] = wpool.tile([128, WH_COLS], BF16,
                                 tag="wh", name=f"wh_e{e}")
            eng.dma_start(out=wh_t[e], in_=whb[e])

        # ensemble 0's weights are on the cold-start critical path: split
        # them over BOTH queues as two separate tiles (layers 0-3 via
        # scalar, 4-6 via sync behind xw) so no single queue's latency
        # jitter can stall hidden(0).
        wh0a = const.tile([128, SPL], BF16)
        wh0b = const.tile([128, WH_COLS - SPL], BF16)

        def _part(e, l):
            if e == 0:
                return (wh0a, 0) if l < 4 else (wh0b, SPL)
            return wh_t[e], 0

        def wh_ap(e, l):
            """(tile, weight column base) for hidden layer l of ensemble e."""
            t, off = _part(e, l)
            base = l * 512 if l < 4 else SPL + (l - 4) * 512
            return t, base - off

        def bh_ap(e, l, oc):
            t, off = _part(e, l)
            bb = (4 * 512 + l * 4 if l < 4
                  else SPL + 3 * 512 + (l - 4) * 4) + 2 * oc - off
            return t[:, bb:bb + 2].bitcast(F32)

        # startup: ONE sync DMA carries x (quad-replicated on host) plus all
        # ten ensembles' L0 weights; the scalar queue concurrently streams
        # e0/e1 hidden weights. Every DMA pays ~4-5us of fixed queue+
        # descriptor latency, so fewer/bigger transfers win.
        nc.sync.dma_start(out=x_t, in_=xw)
        nc.scalar.dma_start(out=wh0a, in_=whb[0][:, :SPL])
        nc.sync.dma_start(out=wh0b, in_=whb[0][:, SPL:])
        issue_wh(1, nc.scalar)
        issue_wh(2, nc.sync)

        # ---- PE pre-warm: dummy matmuls on zeroed scratch, result unread ----
        for k in range(N_WARM):
            ps_warm = pspool.tile([128, 512], F32, tag="ps", name=f"warm{k}")
            nc.tensor.matmul(ps_warm, lhsT=scratch[:, 0:128],
                             rhs=scratch, start=True, stop=True)

        def relu_tile(engine_is_act, dst, ps, bias_ap):
            if engine_is_act:
                nc.scalar.activation(out=dst, in_=ps, func=relu,
                                     bias=bias_ap if bias_ap is not None else 0.0)
            elif bias_ap is not None:
                nc.vector.tensor_scalar(out=dst, in0=ps, scalar1=bias_ap,
                                        scalar2=0.0, op0=add, op1=mx)
            else:
                nc.vector.tensor_scalar(out=dst, in0=ps, scalar1=0.0,
                                        scalar2=None, op0=mx)

        def issue_l0(e):
            # x^T (7, BC) -> h (2x128, BC); bias folded in. 4 K=7 matmuls run
            # concurrently in the 4 PE row groups (row-group i holds weights
            # for oc=i//2, streams bt parity i%2). Relus per 256-col slice,
            # alternated ACT/DVE.
            l0_out[e] = [l0pool.tile([128, BC], BF16, tag=f"l0_{kc}",
                                     name=f"l0_{kc}_e{e}")
                         for kc in range(2)]
            for j in range(2):
                pst = {}
                for i in range(4):
                    p = i % 2
                    bt = 2 * j + p
                    pst[i] = pspool.tile([128, 512], F32, tag="ps",
                                         name=f"ps{i}_{j}_e{e}L0")
                    w0c = BC + e * 128
                    nc.tensor.matmul(
                        pst[i],
                        lhsT=x_t[32 * i:32 * i + IN_DIM + 1, w0c:w0c + 128],
                        rhs=x_t[32 * i:32 * i + IN_DIM + 1,
                                bt * 512:(bt + 1) * 512],
                        start=True, stop=True, tile_position=(32 * i, 0))
                for i in range(4):
                    oc, p = i // 2, i % 2
                    bt = 2 * j + p
                    for s in range(2):
                        c0 = bt * 512 + s * 256
                        relu_tile(s == 0, l0_out[e][oc][:, c0:c0 + 256],
                                  pst[i][:, s * 256:(s + 1) * 256], None)

        issue_l0(0)

        def issue_hidden(e, h_cur, l_lo, l_hi):
            # hidden layers [l_lo, l_hi): K=256 (2 chunks), M=256 (2 chunks)
            for l in range(l_lo, l_hi):
                if l < N_HID - 1:
                    h_nxt = [hpool.tile([128, BC], BF16, tag=f"h{kc}",
                                        name=f"h{kc}_e{e}l{l}")
                             for kc in range(2)]
                else:
                    h_nxt = [hfpool.tile([128, BC], BF16, tag="hf",
                                         name=f"hf{e}_{kc}")
                             for kc in range(2)]
                    for kc in range(2):
                        hf[(e, kc)] = h_nxt[kc]
                wt, base = wh_ap(e, l)
                eng = 0
                for bt in range(4):
                    hsl = slice(bt * 512, (bt + 1) * 512)
                    for oc in range(2):
                        ps = pspool.tile([128, 512], F32, tag="ps",
                                         name=f"ps{oc}_{bt}_e{e}l{l}")
                        c0 = base + oc * 128
                        c1 = base + N_UNITS + oc * 128
                        nc.tensor.matmul(ps, lhsT=wt[:, c0:c0 + 128],
                                         rhs=h_cur[0][:, hsl],
                                         start=True, stop=False)
                        nc.tensor.matmul(ps, lhsT=wt[:, c1:c1 + 128],
                                         rhs=h_cur[1][:, hsl],
                                         start=False, stop=True)
                        relu_tile(eng % 2 == 0, h_nxt[oc][:, hsl], ps,
                                  bh_ap(e, l, oc))
                        eng += 1
                h_cur = h_nxt
            return h_cur

        for e in range(ENS):
            if e + 3 < ENS:
                # prefetch three deep, alternating queues by parity, to
                # absorb the per-DMA queue latency.
                issue_wh(e + 3, nc.scalar if (e + 3) % 2 else nc.sync)
            if e == 1:
                nc.sync.dma_start(out=v_all, in_=vwp)
            # L0(e+1) is issued BETWEEN hidden layers 4 and 5 of ensemble e:
            # its relus then order ahead of the tail-layer relus in the
            # ACT/DVE queues, avoiding a priority inversion where the PE
            # stalls on an L0-relu PSUM release queued behind an l6 relu.
            h_cur = issue_hidden(e, l0_out.pop(e), 0, N_HID - 2)
            if e + 1 < ENS:
                issue_l0(e + 1)
            issue_hidden(e, h_cur, N_HID - 2, N_HID)

        # ---- final layer: out[o', b] = sum_e sum_kc V[e][kc].T @ hf[e][kc] ----
        # Ensemble sum accumulates in PSUM (20 matmuls per bank). Bias-add is
        # split 256/256 across ACT+DVE; stores split across scalar/sync.
        for g, (bt, oc) in enumerate([(bt, oc) for bt in range(4)
                                      for oc in range(2)]):
            ps = pspool.tile([128, 512], F32, tag="ps", name=f"psf{g}")
            hsl = slice(bt * 512, (bt + 1) * 512)
            for e in range(ENS):
                for kc in range(2):
                    c = e * 512 + kc * 256 + oc * 128
                    nc.tensor.matmul(ps, lhsT=v_all[:, c:c + 128],
                                     rhs=hf[(e, kc)][:, hsl],
                                     start=(e == 0 and kc == 0),
                                     stop=(e == ENS - 1 and kc == 1))
            stage = spool.tile([128, 512], F32, tag="s", name=f"stage{g}")
            bpc = ENS * 512 + 2 * oc
            bp_ap = v_all[:, bpc:bpc + 2].bitcast(F32)
            nc.scalar.activation(out=stage[:, 0:256], in_=ps[:, 0:256],
                                 func=ident, bias=bp_ap)
            nc.vector.tensor_scalar(out=stage[:, 256:512], in0=ps[:, 256:512],
                                    scalar1=bp_ap,
                                    scalar2=None, op0=add)
            # partition-split stores: descriptor count (and so DMA latency)
            # scales with partition rows, and the LAST store's latency sits
            # on the kernel's critical path before the exit drain.
            nc.scalar.dma_start(out=yt[oc * 128:oc * 128 + 64, hsl],
                                in_=stage[0:64, :])
            nc.sync.dma_start(out=yt[oc * 128 + 64:(oc + 1) * 128, hsl],
                              in_=stage[64:128, :])

    nc.compile()
    return nc


def prepare_inputs(x, W0, b0, Wh, bh, Wf, bf):
    """Host-side weight refactoring + per-core sharding. Exact fp32 linear
    algebra for the folds; bf16 quantization only at the very end."""
    x = np.asarray(x, np.float32)
    W0 = np.asarray(W0, np.float32)
    b0 = np.asarray(b0, np.float32)
    Wh = np.asarray(Wh, np.float32)
    bh = np.asarray(bh, np.float32)
    Wf = np.asarray(Wf, np.float32)
    bf = np.asarray(bf, np.float32)

    # layer 0 with bias folded: lhsT rows = 6 inputs + ones row; packed into
    # the 4 PE row groups (groups 0,1 -> oc0 weights; groups 2,3 -> oc1).
    w0a = np.concatenate([W0, b0[:, None, :]], axis=1)  # (ENS, 7, 256)
    w0q = np.zeros((128, ENS, 128), np.float32)
    for i in range(4):
        w0q[32 * i:32 * i + IN_DIM + 1] = \
            w0a[:, :, (i // 2) * 128:(i // 2) * 128 + 128].transpose(1, 0, 2)
    w0q = w0q.reshape(128, ENS * 128)

    # hidden weights -> [e, p, (l, kc, o)] with the biases [e, p, (l, oc)]
    # (as bf16) appended so each ensemble is ONE dma.
    whh = (Wh.transpose(1, 0, 2, 3)              # (e, l, h, o)
             .reshape(ENS, N_HID, 2, 128, N_UNITS)
             .transpose(0, 3, 1, 2, 4)           # (e, p, l, kc, o)
             .reshape(ENS, 128, N_HID * 2 * N_UNITS))
    bhh = (bh.transpose(1, 0, 2)                 # (e, l, o)
             .reshape(ENS, N_HID, 2, 128)
             .transpose(0, 3, 1, 2)              # (e, p, l, oc)
             .reshape(ENS, 128, N_HID * 2))
    whh16 = np.ascontiguousarray(whh).astype(BF16_NP).view('<u2')
    bhh16 = np.ascontiguousarray(bhh.astype('<f4')).view('<u2')  # raw bytes
    # [l0-3 w | l0-3 biases | l4-6 w | l4-6 biases] so ensemble 0 can split
    # into two tiles with each half carrying its own biases
    whb = np.concatenate([whh16[:, :, :4 * 512], bhh16[:, :, :16],
                          whh16[:, :, 4 * 512:], bhh16[:, :, 16:]],
                         axis=2)                 # (ENS, 128, 3612) u2
    whb = np.ascontiguousarray(whb).view(BF16_NP)

    # fold the strided channel-mix mean into the final weights:
    # out[b, o'] = 0.1 * sum_j yflat[b, o'*10+j],  yflat col c = e*201+o
    C = ENS * OUT_DIM
    M = np.zeros((C, OUT_DIM), np.float32)
    M[np.arange(C), np.arange(C) // ENS] = 1.0 / ENS
    Me = M.reshape(ENS, OUT_DIM, OUT_DIM)
    V = np.einsum('eho,eoc->ehc', Wf, Me)        # (ENS, 256, 201)
    bpv = bf.reshape(C) @ M                      # (201,)

    Vp = np.zeros((ENS, N_UNITS, 256), np.float32)
    Vp[:, :, :OUT_DIM] = V
    vww = (Vp.reshape(ENS, 2, 128, 256)
             .transpose(0, 2, 1, 3)              # (e, p, kc, o')
             .reshape(ENS, 128, 2 * 256)
             .transpose(1, 0, 2)
             .reshape(128, ENS * 512))
    bp_pad = np.zeros(256, np.float32)
    bp_pad[:OUT_DIM] = bpv
    bp_t = np.ascontiguousarray(bp_pad.reshape(2, 128).T.astype('<f4'))
    vwp = np.concatenate(
        [np.ascontiguousarray(vww).astype(BF16_NP).view('<u2'),
         bp_t.view('<u2')], axis=1)              # (128, 5124) u2
    vwp = np.ascontiguousarray(vwp).view(BF16_NP)

    ones = np.ones((1, BC), np.float32)
    in_maps = []
    for c in range(N_CORES):
        xs = x[c * BC:(c + 1) * BC]              # (BC, 6)
        xt = np.concatenate([xs.T, ones], axis=0)  # (7, BC)
        xrep = np.zeros((128, BC), np.float32)
        for i in range(4):
            xrep[32 * i:32 * i + IN_DIM + 1, :] = xt
        xwc = np.concatenate([xrep, w0q], axis=1)  # (128, BC + 1280)
        xwc = np.ascontiguousarray(xwc).astype(BF16_NP)
        in_maps.append({"xw": xwc, "whb": whb, "vwp": vwp})
    return in_maps


def run(in_maps, trace=False, tmpdir=None):
    if "nc" not in _CACHE:
        _CACHE["nc"] = build_program()
    nc = _CACHE["nc"]
    res = bass_utils.run_bass_kernel_spmd(
        nc, in_maps, core_ids=list(range(N_CORES)), trace=trace, tmpdir=tmpdir)
    return res


def kernel(x, W0, b0, Wh, bh, Wf, bf):
    in_maps = prepare_inputs(x, W0, b0, Wh, bh, Wf, bf)
    res = run(in_maps)
    out = np.empty((BATCH, OUT_DIM), np.float32)
    for c in range(N_CORES):
        out[c * BC:(c + 1) * BC, :] = res.results[c]["yt"][:OUT_DIM].T
    return out


# revision 70
# speedup vs baseline: 1.0437x; 1.0026x over previous
"""Ensemble-MLP (grouped 1x1 conv) Trainium2 kernel.

Computation (per batch row b):
  h = relu(x @ W0[e] + b0[e])             e = 0..9 ensembles, 256 units
  h = relu(h @ Wh[l,e] + bh[l,e])         l = 0..6 hidden layers
  y[e] = h @ Wf[e] + bf[e]                201 outputs per ensemble
  out[b, o'] = mean_j yflat[b, o'*10 + j] (strided channel mix, yflat = e*201+o)

Strategy (v3):
  * Data parallel: batch 16384 -> 2048 rows per core on 8 cores. Weights
    replicated, no collectives.
  * bf16 operands on the PE (fp8 blows the 2e-2 error budget: ~15% measured);
    fp32 PSUM accumulation.
  * Activations live transposed in SBUF: H[channel, batch], 2 chunks of 128
    channels x 2048 batch. Every layer: matmul(psum[o,b] += W[c,o].T @ H[c,b]).
  * Layer-0 bias folded into the matmul via an all-ones row on x^T (K=7).
    x ships as a tiny (8, 2048) tensor and is quad-replicated on-chip into
    partition offsets 0/32/64/96 so L0's K=7 matmuls run 4-concurrent in the
    PE row groups. Hidden biases ride the relu post-op.
  * L0 of ensemble e+1 is issued BEFORE the hidden layers of ensemble e:
    relu/semaphore latency at every layer-0 boundary is hidden behind PE work.
  * All DMAs ride the sync+scalar HW-DGE queues. gpsimd issues nothing:
    its end-of-kernel software-DGE drain (~8us) disappears.
  * Weight DMAs are issued two ensembles ahead (triple-buffered) so
    LDWEIGHTS never waits on the weight queue.
  * Final channel-mixing mean folded into the last-layer weights on the host
    (exact linear algebra). Layer-7 activations for all 10 ensembles are kept
    in SBUF (bf16, 10MB) and the ensemble sum accumulates IN PSUM (20 matmuls
    per bank slice). Bias-add split 256/256 across ACT+DVE, stores split
    across the scalar/sync queues, so the post-matmul tail is minimal.
  * PE pre-warm: dummy matmuls on a memset scratch tile right after the
    framework preamble so the DVFS ramp starts before real work.
"""

import numpy as np
from contextlib import ExitStack

import ml_dtypes
import concourse.bass as bass
import concourse.mybir as mybir
import concourse.tile as tile
from concourse import bacc, bass_utils

F32 = mybir.dt.float32
BF16 = mybir.dt.bfloat16
BF16_NP = ml_dtypes.bfloat16

ENS, N_UNITS, N_HID, IN_DIM, OUT_DIM, BATCH = 10, 256, 7, 6, 201, 16384
N_CORES = 8
BC = BATCH // N_CORES          # 2048 batch rows per core
N_WARM = 10                    # pre-warm matmuls for the DVFS ramp

_CACHE = {}


def build_program():
    nc = bacc.Bacc("TRN2", debug=False)

    # Every [128, *] DMA costs ~4.2us in descriptor generation (128 rows x
    # ~33ns) regardless of size, so tensors are MERGED into as few DMAs as
    # possible: xw ships x + all L0 weights on 8 partitions (8 descriptors!)
    # and is quad-replicated on-chip; whb carries a whole ensemble's hidden
    # weights + biases (bf16) in one DMA; vwp carries the final weights +
    # bias in one DMA.
    XW_COLS = BC + ENS * 128
    xw = nc.dram_tensor("xw", (128, XW_COLS), BF16, kind="ExternalInput").ap()
    # biases ride along as raw f32 bytes (2 bf16 slots each), bitcast on read
    whb = nc.dram_tensor("whb", (ENS, 128, N_HID * 512 + N_HID * 4), BF16,
                         kind="ExternalInput").ap()
    vwp = nc.dram_tensor("vwp", (128, ENS * 512 + 4), BF16,
                         kind="ExternalInput").ap()
    yt = nc.dram_tensor("yt", (256, BC), F32, kind="ExternalOutput").ap()

    add = mybir.AluOpType.add
    mx = mybir.AluOpType.max
    relu = mybir.ActivationFunctionType.Relu
    ident = mybir.ActivationFunctionType.Identity

    with ExitStack() as ctx:
        tc = ctx.enter_context(tile.TileContext(nc))
        const = ctx.enter_context(tc.tile_pool(name="const", bufs=1))
        wpool = ctx.enter_context(tc.tile_pool(name="w", bufs=4))
        vpool = ctx.enter_context(tc.tile_pool(name="v", bufs=1))
        l0pool = ctx.enter_context(tc.tile_pool(name="l0", bufs=2))
        hpool = ctx.enter_context(tc.tile_pool(name="h", bufs=2))
        hfpool = ctx.enter_context(tc.tile_pool(name="hf", bufs=2 * ENS))
        spool = ctx.enter_context(tc.tile_pool(name="stage", bufs=3))
        pspool = ctx.enter_context(tc.tile_pool(name="ps", bufs=8, space="PSUM"))

        x_t = const.tile([128, XW_COLS], BF16)
        scratch = const.tile([128, 512], BF16)
        v_all = vpool.tile([128, ENS * 512 + 4], BF16)

        wh_t = {}
        l0_out, hf = {}, {}

        # memset on gpsimd: vector/scalar are stuck behind their framework
        # table-load DMAs at startup; gpsimd is free ~1us earlier. gpsimd
        # issues no DMAs, so no software-DGE drain at kernel end.
        nc.gpsimd.memset(scratch, 0.0)

        # whb column layout (per ensemble, built on host):
        #   [l0-3 weights 2048 | l0-3 biases 16 | l4-6 weights 1536 |
        #    l4-6 biases 12]  -> 3612 bf16 cols; biases are raw f32 bytes.
        WH_COLS = N_HID * 512 + N_HID * 4
        SPL = 4 * 512 + 4 * 4  # 2064: start of the l4-6 half

        def issue_wh(e, eng):
            wh_t[e] = wpool.tile([128, WH_COLS], BF16,
                                 tag="wh", name=f"wh_e{e}")
            eng.dma_start(out=wh_t[e], in_=whb[e])

        # ensemble 0's weights are on the cold-start critical path: split
        # them over BOTH queues as two separate tiles (layers 0-3 via
        # scalar, 4-6 via sync behind xw) so no single queue's latency
        # jitter can stall hidden(0).
        wh0a = const.tile([128, SPL], BF16)
        wh0b = const.tile([128, WH_COLS - SPL], BF16)

        def _part(e, l):
            if e == 0:
                return (wh0a, 0) if l < 4 else (wh0b, SPL)
            return wh_t[e], 0

        def wh_ap(e, l):
            """(tile, weight column base) for hidden layer l of ensemble e."""
            t, off = _part(e, l)
            base = l * 512 if l < 4 else SPL + (l - 4) * 512
            return t, base - off

        def bh_ap(e, l, oc):
            t, off = _part(e, l)
            bb = (4 * 512 + l * 4 if l < 4
                  else SPL + 3 * 512 + (l - 4) * 4) + 2 * oc - off
            return t[:, bb:bb + 2].bitcast(F32)

        # startup: ONE sync DMA carries x (quad-replicated on host) plus all
        # ten ensembles' L0 weights; the scalar queue concurrently streams
        # e0/e1 hidden weights. Every DMA pays ~4-5us of fixed queue+
        # descriptor latency, so fewer/bigger transfers win.
        nc.sync.dma_start(out=x_t, in_=xw)
        nc.scalar.dma_start(out=wh0a, in_=whb[0][:, :SPL])
        nc.sync.dma_start(out=wh0b, in_=whb[0][:, SPL:])
        issue_wh(1, nc.scalar)
        issue_wh(2, nc.sync)

        # ---- PE pre-warm: dummy matmuls on zeroed scratch, result unread ----
        for k in range(N_WARM):
            ps_warm = pspool.tile([128, 512], F32, tag="ps", name=f"warm{k}")
            nc.tensor.matmul(ps_warm, lhsT=scratch[:, 0:128],
                             rhs=scratch, start=True, stop=True)

        def relu_tile(engine_is_act, dst, ps, bias_ap):
            if engine_is_act:
                nc.scalar.activation(out=dst, in_=ps, func=relu,
                                     bias=bias_ap if bias_ap is not None else 0.0)
            elif bias_ap is not None:
                nc.vector.tensor_scalar(out=dst, in0=ps, scalar1=bias_ap,
                                        scalar2=0.0, op0=add, op1=mx)
            else:
                nc.vector.tensor_scalar(out=dst, in0=ps, scalar1=0.0,
                                        scalar2=None, op0=mx)

        def issue_l0(e):
            # x^T (7, BC) -> h (2x128, BC); bias folded in. 4 K=7 matmuls run
            # concurrently in the 4 PE row groups (row-group i holds weights
            # for oc=i//2, streams bt parity i%2). Relus per 256-col slice,
            # alternated ACT/DVE.
            l0_out[e] = [l0pool.tile([128, BC], BF16, tag=f"l0_{kc}",
                                     name=f"l0_{kc}_e{e}")
                         for kc in range(2)]
            for j in range(2):
                pst = {}
                for i in range(4):
                    p = i % 2
                    bt = 2 * j + p
                    pst[i] = pspool.tile([128, 512], F32, tag="ps",
                                         name=f"ps{i}_{j}_e{e}L0")
                    w0c = BC + e * 128
                    nc.tensor.matmul(
                        pst[i],
                        lhsT=x_t[32 * i:32 * i + IN_DIM + 1, w0c:w0c + 128],
                        rhs=x_t[32 * i:32 * i + IN_DIM + 1,
                                bt * 512:(bt + 1) * 512],
                        start=True, stop=True, tile_position=(32 * i, 0))
                for i in range(4):
                    oc, p = i // 2, i % 2
                    bt = 2 * j + p
                    for s in range(2):
                        c0 = bt * 512 + s * 256
                        relu_tile(s == 0, l0_out[e][oc][:, c0:c0 + 256],
                                  pst[i][:, s * 256:(s + 1) * 256], None)

        issue_l0(0)
        # filler warms: L0(0) finishes ~1us before e0's hidden weights land
        # on the cold scalar queue; keep the PE busy so the DVFS ramp never
        # resets (full speed needs ~3us of gap-free activity).
        for k in range(3):
            ps_fill = pspool.tile([128, 512], F32, tag="ps", name=f"fill{k}")
            nc.tensor.matmul(ps_fill, lhsT=scratch[:, 0:128],
                             rhs=scratch, start=True, stop=True)

        def issue_hidden(e, h_cur, l_lo, l_hi):
            # hidden layers [l_lo, l_hi): K=256 (2 chunks), M=256 (2 chunks)
            for l in range(l_lo, l_hi):
                if l < N_HID - 1:
                    h_nxt = [hpool.tile([128, BC], BF16, tag=f"h{kc}",
                                        name=f"h{kc}_e{e}l{l}")
                             for kc in range(2)]
                else:
                    h_nxt = [hfpool.tile([128, BC], BF16, tag="hf",
                                         name=f"hf{e}_{kc}")
                             for kc in range(2)]
                    for kc in range(2):
                        hf[(e, kc)] = h_nxt[kc]
                wt, base = wh_ap(e, l)
                eng = 0
                for bt in range(4):
                    hsl = slice(bt * 512, (bt + 1) * 512)
                    for oc in range(2):
                        ps = pspool.tile([128, 512], F32, tag="ps",
                                         name=f"ps{oc}_{bt}_e{e}l{l}")
                        c0 = base + oc * 128
                        c1 = base + N_UNITS + oc * 128
                        nc.tensor.matmul(ps, lhsT=wt[:, c0:c0 + 128],
                                         rhs=h_cur[0][:, hsl],
                                         start=True, stop=False)
                        nc.tensor.matmul(ps, lhsT=wt[:, c1:c1 + 128],
                                         rhs=h_cur[1][:, hsl],
                                         start=False, stop=True)
                        relu_tile(eng % 2 == 0, h_nxt[oc][:, hsl], ps,
                                  bh_ap(e, l, oc))
                        eng += 1
                h_cur = h_nxt
            return h_cur

        for e in range(ENS):
            if e + 3 < ENS:
                # prefetch three deep, alternating queues by parity, to
                # absorb the per-DMA queue latency.
                issue_wh(e + 3, nc.scalar if (e + 3) % 2 else nc.sync)
            if e == 1:
                nc.sync.dma_start(out=v_all, in_=vwp)
            # L0(e+1) is issued BETWEEN hidden layers 4 and 5 of ensemble e:
            # its relus then order ahead of the tail-layer relus in the
            # ACT/DVE queues, avoiding a priority inversion where the PE
            # stalls on an L0-relu PSUM release queued behind an l6 relu.
            h_cur = issue_hidden(e, l0_out.pop(e), 0, N_HID - 2)
            if e + 1 < ENS:
                issue_l0(e + 1)
            issue_hidden(e, h_cur, N_HID - 2, N_HID)

        # ---- final layer: out[o', b] = sum_e sum_kc V[e][kc].T @ hf[e][kc] ----
        # Ensemble sum accumulates in PSUM (20 matmuls per bank). Bias-add is
        # split 256/256 across ACT+DVE; stores split across scalar/sync.
        for g, (bt, oc) in enumerate([(bt, oc) for bt in range(4)
                                      for oc in range(2)]):
            ps = pspool.tile([128, 512], F32, tag="ps", name=f"psf{g}")
            hsl = slice(bt * 512, (bt + 1) * 512)
            for e in range(ENS):
                for kc in range(2):
                    c = e * 512 + kc * 256 + oc * 128
                    nc.tensor.matmul(ps, lhsT=v_all[:, c:c + 128],
                                     rhs=hf[(e, kc)][:, hsl],
                                     start=(e == 0 and kc == 0),
                                     stop=(e == ENS - 1 and kc == 1))
            stage = spool.tile([128, 512], F32, tag="s", name=f"stage{g}")
            bpc = ENS * 512 + 2 * oc
            bp_ap = v_all[:, bpc:bpc + 2].bitcast(F32)
            nc.scalar.activation(out=stage[:, 0:256], in_=ps[:, 0:256],
                                 func=ident, bias=bp_ap)
            nc.vector.tensor_scalar(out=stage[:, 256:512], in0=ps[:, 256:512],
                                    scalar1=bp_ap,
                                    scalar2=None, op0=add)
            # partition-split stores: descriptor count (and so DMA latency)
            # scales with partition rows, and the LAST store's latency sits
            # on the kernel's critical path before the exit drain.
            nc.scalar.dma_start(out=yt[oc * 128:oc * 128 + 64, hsl],
                                in_=stage[0:64, :])
            nc.sync.dma_start(out=yt[oc * 128 + 64:(oc + 1) * 128, hsl],
                              in_=stage[64:128, :])

    nc.compile()
    return nc


def prepare_inputs(x, W0, b0, Wh, bh, Wf, bf):
    """Host-side weight refactoring + per-core sharding. Exact fp32 linear
    algebra for the folds; bf16 quantization only at the very end."""
    x = np.asarray(x, np.float32)
    W0 = np.asarray(W0, np.float32)
    b0 = np.asarray(b0, np.float32)
    Wh = np.asarray(Wh, np.float32)
    bh = np.asarray(bh, np.float32)
    Wf = np.asarray(Wf, np.float32)
    bf = np.asarray(bf, np.float32)

    # layer 0 with bias folded: lhsT rows = 6 inputs + ones row; packed into
    # the 4 PE row groups (groups 0,1 -> oc0 weights; groups 2,3 -> oc1).
    w0a = np.concatenate([W0, b0[:, None, :]], axis=1)  # (ENS, 7, 256)
    w0q = np.zeros((128, ENS, 128), np.float32)
    for i in range(4):
        w0q[32 * i:32 * i + IN_DIM + 1] = \
            w0a[:, :, (i // 2) * 128:(i // 2) * 128 + 128].transpose(1, 0, 2)
    w0q = w0q.reshape(128, ENS * 128)

    # hidden weights -> [e, p, (l, kc, o)] with the biases [e, p, (l, oc)]
    # (as bf16) appended so each ensemble is ONE dma.
    whh = (Wh.transpose(1, 0, 2, 3)              # (e, l, h, o)
             .reshape(ENS, N_HID, 2, 128, N_UNITS)
             .transpose(0, 3, 1, 2, 4)           # (e, p, l, kc, o)
             .reshape(ENS, 128, N_HID * 2 * N_UNITS))
    bhh = (bh.transpose(1, 0, 2)                 # (e, l, o)
             .reshape(ENS, N_HID, 2, 128)
             .transpose(0, 3, 1, 2)              # (e, p, l, oc)
             .reshape(ENS, 128, N_HID * 2))
    whh16 = np.ascontiguousarray(whh).astype(BF16_NP).view('<u2')
    bhh16 = np.ascontiguousarray(bhh.astype('<f4')).view('<u2')  # raw bytes
    # [l0-3 w | l0-3 biases | l4-6 w | l4-6 biases] so ensemble 0 can split
    # into two tiles with each half carrying its own biases
    whb = np.concatenate([whh16[:, :, :4 * 512], bhh16[:, :, :16],
                          whh16[:, :, 4 * 512:], bhh16[:, :, 16:]],
                         axis=2)                 # (ENS, 128, 3612) u2
    whb = np.ascontiguousarray(whb).view(BF16_NP)

    # fold the strided channel-mix mean into the final weights:
    # out[b, o'] = 0.1 * sum_j yflat[b, o'*10+j],  yflat col c = e*201+o
    C = ENS * OUT_DIM
    M = np.zeros((C, OUT_DIM), np.float32)
    M[np.arange(C), np.arange(C) // ENS] = 1.0 / ENS
    Me = M.reshape(ENS, OUT_DIM, OUT_DIM)
    V = np.einsum('eho,eoc->ehc', Wf, Me)        # (ENS, 256, 201)
    bpv = bf.reshape(C) @ M                      # (201,)

    Vp = np.zeros((ENS, N_UNITS, 256), np.float32)
    Vp[:, :, :OUT_DIM] = V
    vww = (Vp.reshape(ENS, 2, 128, 256)
             .transpose(0, 2, 1, 3)              # (e, p, kc, o')
             .reshape(ENS, 128, 2 * 256)
             .transpose(1, 0, 2)
             .reshape(128, ENS * 512))
    bp_pad = np.zeros(256, np.float32)
    bp_pad[:OUT_DIM] = bpv
    bp_t = np.ascontiguousarray(bp_pad.reshape(2, 128).T.astype('<f4'))
    vwp = np.concatenate(
        [np.ascontiguousarray(vww).astype(BF16_NP).view('<u2'),
         bp_t.view('<u2')], axis=1)              # (128, 5124) u2
    vwp = np.ascontiguousarray(vwp).view(BF16_NP)

    ones = np.ones((1, BC), np.float32)
    in_maps = []
    for c in range(N_CORES):
        xs = x[c * BC:(c + 1) * BC]              # (BC, 6)
        xt = np.concatenate([xs.T, ones], axis=0)  # (7, BC)
        xrep = np.zeros((128, BC), np.float32)
        for i in range(4):
            xrep[32 * i:32 * i + IN_DIM + 1, :] = xt
        xwc = np.concatenate([xrep, w0q], axis=1)  # (128, BC + 1280)
        xwc = np.ascontiguousarray(xwc).astype(BF16_NP)
        in_maps.append({"xw": xwc, "whb": whb, "vwp": vwp})
    return in_maps


def run(in_maps, trace=False, tmpdir=None):
    if "nc" not in _CACHE:
        _CACHE["nc"] = build_program()
    nc = _CACHE["nc"]
    res = bass_utils.run_bass_kernel_spmd(
        nc, in_maps, core_ids=list(range(N_CORES)), trace=trace, tmpdir=tmpdir)
    return res


def kernel(x, W0, b0, Wh, bh, Wf, bf):
    in_maps = prepare_inputs(x, W0, b0, Wh, bh, Wf, bf)
    res = run(in_maps)
    out = np.empty((BATCH, OUT_DIM), np.float32)
    for c in range(N_CORES):
        out[c * BC:(c + 1) * BC, :] = res.results[c]["yt"][:OUT_DIM].T
    return out


# revision 73
# speedup vs baseline: 1.0503x; 1.0063x over previous
"""Ensemble-MLP (grouped 1x1 conv) Trainium2 kernel.

Computation (per batch row b):
  h = relu(x @ W0[e] + b0[e])             e = 0..9 ensembles, 256 units
  h = relu(h @ Wh[l,e] + bh[l,e])         l = 0..6 hidden layers
  y[e] = h @ Wf[e] + bf[e]                201 outputs per ensemble
  out[b, o'] = mean_j yflat[b, o'*10 + j] (strided channel mix, yflat = e*201+o)

Strategy (v3):
  * Data parallel: batch 16384 -> 2048 rows per core on 8 cores. Weights
    replicated, no collectives.
  * bf16 operands on the PE (fp8 blows the 2e-2 error budget: ~15% measured);
    fp32 PSUM accumulation.
  * Activations live transposed in SBUF: H[channel, batch], 2 chunks of 128
    channels x 2048 batch. Every layer: matmul(psum[o,b] += W[c,o].T @ H[c,b]).
  * Layer-0 bias folded into the matmul via an all-ones row on x^T (K=7).
    x ships as a tiny (8, 2048) tensor and is quad-replicated on-chip into
    partition offsets 0/32/64/96 so L0's K=7 matmuls run 4-concurrent in the
    PE row groups. Hidden biases ride the relu post-op.
  * L0 of ensemble e+1 is issued BEFORE the hidden layers of ensemble e:
    relu/semaphore latency at every layer-0 boundary is hidden behind PE work.
  * All DMAs ride the sync+scalar HW-DGE queues. gpsimd issues nothing:
    its end-of-kernel software-DGE drain (~8us) disappears.
  * Weight DMAs are issued two ensembles ahead (triple-buffered) so
    LDWEIGHTS never waits on the weight queue.
  * Final channel-mixing mean folded into the last-layer weights on the host
    (exact linear algebra). Layer-7 activations for all 10 ensembles are kept
    in SBUF (bf16, 10MB) and the ensemble sum accumulates IN PSUM (20 matmuls
    per bank slice). Bias-add split 256/256 across ACT+DVE, stores split
    across the scalar/sync queues, so the post-matmul tail is minimal.
  * PE pre-warm: dummy matmuls on a memset scratch tile right after the
    framework preamble so the DVFS ramp starts before real work.
"""

import numpy as np
from contextlib import ExitStack

import ml_dtypes
import concourse.bass as bass
import concourse.mybir as mybir
import concourse.tile as tile
from concourse import bacc, bass_utils

F32 = mybir.dt.float32
BF16 = mybir.dt.bfloat16
BF16_NP = ml_dtypes.bfloat16

ENS, N_UNITS, N_HID, IN_DIM, OUT_DIM, BATCH = 10, 256, 7, 6, 201, 16384
N_CORES = 8
BC = BATCH // N_CORES          # 2048 batch rows per core
N_WARM = 10                    # pre-warm matmuls for the DVFS ramp

_CACHE = {}


def build_program():
    nc = bacc.Bacc("TRN2", debug=False)

    # Every [128, *] DMA costs ~4.2us in descriptor generation (128 rows x
    # ~33ns) regardless of size, so tensors are MERGED into as few DMAs as
    # possible: xw ships x + all L0 weights on 8 partitions (8 descriptors!)
    # and is quad-replicated on-chip; whb carries a whole ensemble's hidden
    # weights + biases (bf16) in one DMA; vwp carries the final weights +
    # bias in one DMA.
    XW_COLS = BC + ENS * 128
    xw = nc.dram_tensor("xw", (128, XW_COLS), BF16, kind="ExternalInput").ap()
    # biases ride along as raw f32 bytes (2 bf16 slots each), bitcast on read
    whb = nc.dram_tensor("whb", (ENS, 128, N_HID * 512 + N_HID * 4), BF16,
                         kind="ExternalInput").ap()
    vwp = nc.dram_tensor("vwp", (128, ENS * 512 + 4), BF16,
                         kind="ExternalInput").ap()
    yt = nc.dram_tensor("yt", (256, BC), F32, kind="ExternalOutput").ap()

    add = mybir.AluOpType.add
    mx = mybir.AluOpType.max
    relu = mybir.ActivationFunctionType.Relu
    ident = mybir.ActivationFunctionType.Identity

    with ExitStack() as ctx:
        tc = ctx.enter_context(tile.TileContext(nc))
        const = ctx.enter_context(tc.tile_pool(name="const", bufs=1))
        wpool = ctx.enter_context(tc.tile_pool(name="w", bufs=4))
        vpool = ctx.enter_context(tc.tile_pool(name="v", bufs=1))
        l0pool = ctx.enter_context(tc.tile_pool(name="l0", bufs=2))
        hpool = ctx.enter_context(tc.tile_pool(name="h", bufs=2))
        hfpool = ctx.enter_context(tc.tile_pool(name="hf", bufs=2 * ENS))
        spool = ctx.enter_context(tc.tile_pool(name="stage", bufs=3))
        pspool = ctx.enter_context(tc.tile_pool(name="ps", bufs=8, space="PSUM"))

        x_t = const.tile([128, XW_COLS], BF16)
        scratch = const.tile([128, 512], BF16)
        v_all = vpool.tile([128, ENS * 512 + 4], BF16)

        wh_t = {}
        l0_out, hf = {}, {}

        # memset on gpsimd: vector/scalar are stuck behind their framework
        # table-load DMAs at startup; gpsimd is free ~1us earlier. gpsimd
        # issues no DMAs, so no software-DGE drain at kernel end.
        nc.gpsimd.memset(scratch, 0.0)

        # whb column layout (per ensemble, built on host), biases as raw f32
        # bytes behind their own layer group:
        #   [l0l1 w 1024 | b 8 | l2l3 w 1024 | b 8 | l4-6 w 1536 | b 12]
        WH_COLS = N_HID * 512 + N_HID * 4
        #       (seg col base, first layer, bias col base, seg col end)
        SEGS = [(0, 0, 1024, 1032),
                (1032, 2, 2056, 2064),
                (2064, 4, 3600, 3612)]

        def issue_wh(e, eng):
            wh_t[e] = wpool.tile([128, WH_COLS], BF16,
                                 tag="wh", name=f"wh_e{e}")
            eng.dma_start(out=wh_t[e], in_=whb[e])

        # ensemble 0's weights are on the cold-start critical path: three
        # separate tiles so hidden(0) layer l only waits its own segment —
        # l0l1 rides at the head of the scalar queue and lands ~1us before
        # the first hidden matmuls need it.
        wh0 = [const.tile([128, s[3] - s[0]], BF16, name=f"wh0_{i}")
               for i, s in enumerate(SEGS)]

        def _part(e, l):
            seg = SEGS[0 if l < 2 else 1 if l < 4 else 2]
            if e == 0:
                return wh0[SEGS.index(seg)], seg[0], seg
            return wh_t[e], 0, seg

        def wh_ap(e, l):
            """(tile, weight column base) for hidden layer l of ensemble e."""
            t, off, seg = _part(e, l)
            return t, seg[0] + (l - seg[1]) * 512 - off

        def bh_ap(e, l, oc):
            t, off, seg = _part(e, l)
            bb = seg[2] + ((l - seg[1]) * 2 + oc) * 2 - off
            return t[:, bb:bb + 2].bitcast(F32)

        # startup: ONE sync DMA carries x (quad-replicated on host) plus all
        # ten ensembles' L0 weights; the scalar queue concurrently streams
        # e0's first layers then e1. Every DMA pays ~4-5us of fixed queue+
        # descriptor latency, so chunks are as few and as late-needed-last
        # as possible.
        nc.sync.dma_start(out=x_t, in_=xw)
        nc.scalar.dma_start(out=wh0[0], in_=whb[0][:, SEGS[0][0]:SEGS[0][3]])
        nc.scalar.dma_start(out=wh0[1], in_=whb[0][:, SEGS[1][0]:SEGS[1][3]])
        nc.sync.dma_start(out=wh0[2], in_=whb[0][:, SEGS[2][0]:SEGS[2][3]])
        issue_wh(1, nc.scalar)
        issue_wh(2, nc.sync)

        # ---- PE pre-warm: dummy matmuls on zeroed scratch, result unread ----
        for k in range(N_WARM):
            ps_warm = pspool.tile([128, 512], F32, tag="ps", name=f"warm{k}")
            nc.tensor.matmul(ps_warm, lhsT=scratch[:, 0:128],
                             rhs=scratch, start=True, stop=True)

        def relu_tile(engine_is_act, dst, ps, bias_ap):
            if engine_is_act:
                nc.scalar.activation(out=dst, in_=ps, func=relu,
                                     bias=bias_ap if bias_ap is not None else 0.0)
            elif bias_ap is not None:
                nc.vector.tensor_scalar(out=dst, in0=ps, scalar1=bias_ap,
                                        scalar2=0.0, op0=add, op1=mx)
            else:
                nc.vector.tensor_scalar(out=dst, in0=ps, scalar1=0.0,
                                        scalar2=None, op0=mx)

        def issue_l0(e):
            # x^T (7, BC) -> h (2x128, BC); bias folded in. 4 K=7 matmuls run
            # concurrently in the 4 PE row groups (row-group i holds weights
            # for oc=i//2, streams bt parity i%2). Relus per 256-col slice,
            # alternated ACT/DVE.
            l0_out[e] = [l0pool.tile([128, BC], BF16, tag=f"l0_{kc}",
                                     name=f"l0_{kc}_e{e}")
                         for kc in range(2)]
            for j in range(2):
                pst = {}
                for i in range(4):
                    p = i % 2
                    bt = 2 * j + p
                    pst[i] = pspool.tile([128, 512], F32, tag="ps",
                                         name=f"ps{i}_{j}_e{e}L0")
                    w0c = BC + e * 128
                    nc.tensor.matmul(
                        pst[i],
                        lhsT=x_t[32 * i:32 * i + IN_DIM + 1, w0c:w0c + 128],
                        rhs=x_t[32 * i:32 * i + IN_DIM + 1,
                                bt * 512:(bt + 1) * 512],
                        start=True, stop=True, tile_position=(32 * i, 0))
                for i in range(4):
                    oc, p = i // 2, i % 2
                    bt = 2 * j + p
                    for s in range(2):
                        c0 = bt * 512 + s * 256
                        relu_tile(s == 0, l0_out[e][oc][:, c0:c0 + 256],
                                  pst[i][:, s * 256:(s + 1) * 256], None)

        issue_l0(0)
        # filler warm: bridges residual jitter between L0(0) finishing and
        # e0's first-layer weights landing, keeping the DVFS ramp alive.
        for k in range(1):
            ps_fill = pspool.tile([128, 512], F32, tag="ps", name=f"fill{k}")
            nc.tensor.matmul(ps_fill, lhsT=scratch[:, 0:128],
                             rhs=scratch, start=True, stop=True)

        def issue_hidden(e, h_cur, l_lo, l_hi):
            # hidden layers [l_lo, l_hi): K=256 (2 chunks), M=256 (2 chunks)
            for l in range(l_lo, l_hi):
                if l < N_HID - 1:
                    h_nxt = [hpool.tile([128, BC], BF16, tag=f"h{kc}",
                                        name=f"h{kc}_e{e}l{l}")
                             for kc in range(2)]
                else:
                    h_nxt = [hfpool.tile([128, BC], BF16, tag="hf",
                                         name=f"hf{e}_{kc}")
                             for kc in range(2)]
                    for kc in range(2):
                        hf[(e, kc)] = h_nxt[kc]
                wt, base = wh_ap(e, l)
                eng = 0
                for bt in range(4):
                    hsl = slice(bt * 512, (bt + 1) * 512)
                    for oc in range(2):
                        ps = pspool.tile([128, 512], F32, tag="ps",
                                         name=f"ps{oc}_{bt}_e{e}l{l}")
                        c0 = base + oc * 128
                        c1 = base + N_UNITS + oc * 128
                        nc.tensor.matmul(ps, lhsT=wt[:, c0:c0 + 128],
                                         rhs=h_cur[0][:, hsl],
                                         start=True, stop=False)
                        nc.tensor.matmul(ps, lhsT=wt[:, c1:c1 + 128],
                                         rhs=h_cur[1][:, hsl],
                                         start=False, stop=True)
                        relu_tile(eng % 2 == 0, h_nxt[oc][:, hsl], ps,
                                  bh_ap(e, l, oc))
                        eng += 1
                h_cur = h_nxt
            return h_cur

        for e in range(ENS):
            if e + 3 < ENS:
                # prefetch three deep, alternating queues by parity, to
                # absorb the per-DMA queue latency.
                issue_wh(e + 3, nc.scalar if (e + 3) % 2 else nc.sync)
            if e == 1:
                nc.sync.dma_start(out=v_all, in_=vwp)
            # L0(e+1) is issued BETWEEN hidden layers 4 and 5 of ensemble e:
            # its relus then order ahead of the tail-layer relus in the
            # ACT/DVE queues, avoiding a priority inversion where the PE
            # stalls on an L0-relu PSUM release queued behind an l6 relu.
            h_cur = issue_hidden(e, l0_out.pop(e), 0, N_HID - 2)
            if e + 1 < ENS:
                issue_l0(e + 1)
            issue_hidden(e, h_cur, N_HID - 2, N_HID)

        # ---- final layer: out[o', b] = sum_e sum_kc V[e][kc].T @ hf[e][kc] ----
        # Ensemble sum accumulates in PSUM (20 matmuls per bank). Bias-add is
        # split 256/256 across ACT+DVE; stores split across scalar/sync.
        for g, (bt, oc) in enumerate([(bt, oc) for bt in range(4)
                                      for oc in range(2)]):
            ps = pspool.tile([128, 512], F32, tag="ps", name=f"psf{g}")
            hsl = slice(bt * 512, (bt + 1) * 512)
            for e in range(ENS):
                for kc in range(2):
                    c = e * 512 + kc * 256 + oc * 128
                    nc.tensor.matmul(ps, lhsT=v_all[:, c:c + 128],
                                     rhs=hf[(e, kc)][:, hsl],
                                     start=(e == 0 and kc == 0),
                                     stop=(e == ENS - 1 and kc == 1))
            stage = spool.tile([128, 512], F32, tag="s", name=f"stage{g}")
            bpc = ENS * 512 + 2 * oc
            bp_ap = v_all[:, bpc:bpc + 2].bitcast(F32)
            nc.scalar.activation(out=stage[:, 0:256], in_=ps[:, 0:256],
                                 func=ident, bias=bp_ap)
            nc.vector.tensor_scalar(out=stage[:, 256:512], in0=ps[:, 256:512],
                                    scalar1=bp_ap,
                                    scalar2=None, op0=add)
            # partition-split stores: descriptor count (and so DMA latency)
            # scales with partition rows, and the LAST store's latency sits
            # on the kernel's critical path before the exit drain.
            nc.scalar.dma_start(out=yt[oc * 128:oc * 128 + 64, hsl],
                                in_=stage[0:64, :])
            nc.sync.dma_start(out=yt[oc * 128 + 64:(oc + 1) * 128, hsl],
                              in_=stage[64:128, :])

    nc.compile()
    return nc


def prepare_inputs(x, W0, b0, Wh, bh, Wf, bf):
    """Host-side weight refactoring + per-core sharding. Exact fp32 linear
    algebra for the folds; bf16 quantization only at the very end."""
    x = np.asarray(x, np.float32)
    W0 = np.asarray(W0, np.float32)
    b0 = np.asarray(b0, np.float32)
    Wh = np.asarray(Wh, np.float32)
    bh = np.asarray(bh, np.float32)
    Wf = np.asarray(Wf, np.float32)
    bf = np.asarray(bf, np.float32)

    # layer 0 with bias folded: lhsT rows = 6 inputs + ones row; packed into
    # the 4 PE row groups (groups 0,1 -> oc0 weights; groups 2,3 -> oc1).
    w0a = np.concatenate([W0, b0[:, None, :]], axis=1)  # (ENS, 7, 256)
    w0q = np.zeros((128, ENS, 128), np.float32)
    for i in range(4):
        w0q[32 * i:32 * i + IN_DIM + 1] = \
            w0a[:, :, (i // 2) * 128:(i // 2) * 128 + 128].transpose(1, 0, 2)
    w0q = w0q.reshape(128, ENS * 128)

    # hidden weights -> [e, p, (l, kc, o)] with the biases [e, p, (l, oc)]
    # (as bf16) appended so each ensemble is ONE dma.
    whh = (Wh.transpose(1, 0, 2, 3)              # (e, l, h, o)
             .reshape(ENS, N_HID, 2, 128, N_UNITS)
             .transpose(0, 3, 1, 2, 4)           # (e, p, l, kc, o)
             .reshape(ENS, 128, N_HID * 2 * N_UNITS))
    bhh = (bh.transpose(1, 0, 2)                 # (e, l, o)
             .reshape(ENS, N_HID, 2, 128)
             .transpose(0, 3, 1, 2)              # (e, p, l, oc)
             .reshape(ENS, 128, N_HID * 2))
    whh16 = np.ascontiguousarray(whh).astype(BF16_NP).view('<u2')
    bhh16 = np.ascontiguousarray(bhh.astype('<f4')).view('<u2')  # raw bytes
    # [l0l1 w | b | l2l3 w | b | l4-6 w | b]: each segment carries its own
    # biases so ensemble 0 can split into three independent tiles
    whb = np.concatenate([whh16[:, :, :1024], bhh16[:, :, :8],
                          whh16[:, :, 1024:2048], bhh16[:, :, 8:16],
                          whh16[:, :, 2048:], bhh16[:, :, 16:]],
                         axis=2)                 # (ENS, 128, 3612) u2
    whb = np.ascontiguousarray(whb).view(BF16_NP)

    # fold the strided channel-mix mean into the final weights:
    # out[b, o'] = 0.1 * sum_j yflat[b, o'*10+j],  yflat col c = e*201+o
    C = ENS * OUT_DIM
    M = np.zeros((C, OUT_DIM), np.float32)
    M[np.arange(C), np.arange(C) // ENS] = 1.0 / ENS
    Me = M.reshape(ENS, OUT_DIM, OUT_DIM)
    V = np.einsum('eho,eoc->ehc', Wf, Me)        # (ENS, 256, 201)
    bpv = bf.reshape(C) @ M                      # (201,)

    Vp = np.zeros((ENS, N_UNITS, 256), np.float32)
    Vp[:, :, :OUT_DIM] = V
    vww = (Vp.reshape(ENS, 2, 128, 256)
             .transpose(0, 2, 1, 3)              # (e, p, kc, o')
             .reshape(ENS, 128, 2 * 256)
             .transpose(1, 0, 2)
             .reshape(128, ENS * 512))
    bp_pad = np.zeros(256, np.float32)
    bp_pad[:OUT_DIM] = bpv
    bp_t = np.ascontiguousarray(bp_pad.reshape(2, 128).T.astype('<f4'))
    vwp = np.concatenate(
        [np.ascontiguousarray(vww).astype(BF16_NP).view('<u2'),
         bp_t.view('<u2')], axis=1)              # (128, 5124) u2
    vwp = np.ascontiguousarray(vwp).view(BF16_NP)

    ones = np.ones((1, BC), np.float32)
    in_maps = []
    for c in range(N_CORES):
        xs = x[c * BC:(c + 1) * BC]              # (BC, 6)
        xt = np.concatenate([xs.T, ones], axis=0)  # (7, BC)
        xrep = np.zeros((128, BC), np.float32)
        for i in range(4):
            xrep[32 * i:32 * i + IN_DIM + 1, :] = xt
        xwc = np.concatenate([xrep, w0q], axis=1)  # (128, BC + 1280)
        xwc = np.ascontiguousarray(xwc).astype(BF16_NP)
        in_maps.append({"xw": xwc, "whb": whb, "vwp": vwp})
    return in_maps


def run(in_maps, trace=False, tmpdir=None):
    if "nc" not in _CACHE:
        _CACHE["nc"] = build_program()
    nc = _CACHE["nc"]
    res = bass_utils.run_bass_kernel_spmd(
        nc, in_maps, core_ids=list(range(N_CORES)), trace=trace, tmpdir=tmpdir)
    return res


def kernel(x, W0, b0, Wh, bh, Wf, bf):
    in_maps = prepare_inputs(x, W0, b0, Wh, bh, Wf, bf)
    res = run(in_maps)
    out = np.empty((BATCH, OUT_DIM), np.float32)
    for c in range(N_CORES):
        out[c * BC:(c + 1) * BC, :] = res.results[c]["yt"][:OUT_DIM].T
    return out
